# revision 22
# baseline (speedup 1.0000x reference)
"""Trainium2 Bass kernel for nn_BigGNN (3-layer dual-graph TransformerConv GNN).

Strategy (8 NeuronCores, SPMD):
  - Dense reformulation: intra-graph convs (6144 edges over 384 nodes) become
    masked dense attention with a host-built additive log-multiplicity mask;
    cross-graph convs are exact dense bipartite attention (full 384x384).
  - Cores 0-3 own 96-row slices of graph 1, cores 4-7 of graph 2. Each of the
    6 sequential stages (3 layers x {intra, cross}) a core computes its 96
    output rows completely (QK^T softmax, alpha@V both heads, skip, leaky),
    then one 8-rank AllGather shares transposed activation slices. K/V
    projections are recomputed per core from the gathered full activations.
  - SPMD uniformity: per-core 0/1 scalars select which gathered half (graph 1
    vs graph 2) feeds K/V each stage; all other asymmetry lives in input data.
  - Biases are folded into the matmuls via an appended ones-row on the
    contraction dim; the attention 1/sqrt(D) scale and the 2-head mean and the
    V-bias are folded into weights / alpha scaling host-side.
  - Final pooling (column means) + 3-layer MLP + sigmoid replicated on all
    cores; core 0's output is returned.
"""

import math
import numpy as np

import concourse.bass as bass
import concourse.bacc as bacc
import concourse.tile as tile
import concourse.mybir as mybir
# ---------------- constants ----------------
D = 300          # feature dim per head
H = 2            # heads
N = 384          # nodes per graph
SL = 96          # dst-row slice per core
NC = 8           # cores
NLAYERS = 3
SCALE = 1.0 / math.sqrt(D)
NEG = -1.0e5     # additive mask for non-edges

# d-contraction chunks (g, count): d index = g*128 + p
CH = [(0, 128), (1, 128), (2, 44)]
# M-tiles over a head's 300 features / over output d
MT = [(0, 128), (1, 128), (2, 44)]

MODE = "bf16"    # "f32" | "bf16"

F32 = mybir.dt.float32


def _sd(mode):
    return mybir.dt.float32 if mode == "f32" else mybir.dt.bfloat16


# ---------------- device program ----------------

def _build_program(mode):
    SD = _sd(mode)
    nc = bacc.Bacc("TRN2", target_bir_lowering=False, debug=False, num_devices=NC)

    def din(name, shape, dt=None):
        return nc.dram_tensor(name, list(shape), dt or SD, kind="ExternalInput").ap()

    ins = {}
    for k in range(6):
        ins[f"wq{k}"] = din(f"wq{k}", (3, 128, H * D))
        ins[f"wk{k}"] = din(f"wk{k}", (3, 128, H * D))
        ins[f"wv{k}"] = din(f"wv{k}", (3, 128, H * D))
        ins[f"ws{k}"] = din(f"ws{k}", (3, 128, D))
    ins["xsrc0"] = din("xsrc0", (3, 128, N))
    ins["xsl0"] = din("xsl0", (3, 128, SL))
    ins["madd"] = din("madd", (SL, N), F32)
    ins["hvalid"] = din("hvalid", (SL, 1), F32)
    ins["sel"] = din("sel", (128, 4), F32)    # own_a, own_b, oth_a, oth_b
    ins["ident"] = din("ident", (SL, SL))
    for k in range(6):
        # cols 0-5: bq_scaled (h*3+t), 6-11: bk, 12-14: bias_out per d-tile
        ins[f"bias{k}"] = din(f"bias{k}", (128, 15), F32)
    ins["mw1a"] = din("mw1a", (3, 128, D), F32)
    ins["mw1b"] = din("mw1b", (3, 128, D), F32)
    ins["mw2"] = din("mw2", (3, 128, D), F32)
    ins["mw3"] = din("mw3", (3, 128, 1), F32)
    ins["mb1"] = din("mb1", (128, 3), F32)
    ins["mb2"] = din("mb2", (128, 3), F32)
    ins["mb3"] = din("mb3", (1, 1), F32)

    p1o = nc.dram_tensor("p1o", [128, 3], F32, kind="ExternalOutput").ap()
    p2o = nc.dram_tensor("p2o", [128, 3], F32, kind="ExternalOutput").ap()
    sigo = nc.dram_tensor("sigo", [1, 1], F32, kind="ExternalOutput").ap()

    with tile.TileContext(nc) as tc:
        with (
            tc.tile_pool(name="cpool", bufs=1) as cpool,
            tc.tile_pool(name="wpool", bufs=2) as wpool,
            tc.tile_pool(name="xpool", bufs=2) as xpool,
            tc.tile_pool(name="proj", bufs=2) as proj,
            tc.tile_pool(name="attn", bufs=2) as attn,
            tc.tile_pool(name="stats", bufs=8) as stats,
            tc.tile_pool(name="ps_pj", bufs=3, space="PSUM") as ps_pj,
            tc.tile_pool(name="ps_lg", bufs=2, space="PSUM") as ps_lg,
            tc.tile_pool(name="ps_tp", bufs=2, space="PSUM") as ps_tp,
            tc.tile_pool(name="ps_op", bufs=1, space="PSUM") as ps_op,
            tc.tile_pool(name="dram", bufs=2, space="DRAM") as dram,
        ):
            # ---- resident constants ----
            ident = cpool.tile([SL, SL], SD, tag="ident")
            nc.sync.dma_start(ident[:], ins["ident"])
            maddt = cpool.tile([SL, N], F32, tag="madd")
            nc.sync.dma_start(maddt[:], ins["madd"])
            hval = cpool.tile([SL, 1], F32, tag="hval")
            nc.sync.dma_start(hval[:], ins["hvalid"])
            selt = cpool.tile([128, 4], F32, tag="sel")
            nc.sync.dma_start(selt[:], ins["sel"])

            # ---- stage-0 activations from inputs ----
            xsrc_in = xpool.tile([128, 3, N], SD, tag="xsrc")
            xsl_in = xpool.tile([128, 3, SL], SD, tag="xsl")
            for g in range(3):
                nc.sync.dma_start(xsrc_in[:, g, :], ins["xsrc0"][g, :, :])
                nc.sync.dma_start(xsl_in[:, g, :], ins["xsl0"][g, :, :])

            xsl_cur = xsl_in
            xall = None

            def load_weights(k):
                w = {}
                for nm, fdim in (("wq", H * D), ("wk", H * D), ("wv", H * D), ("ws", D)):
                    t = wpool.tile([128, 3, fdim], SD, tag=nm)
                    for g in range(3):
                        nc.sync.dma_start(t[:, g, :], ins[f"{nm}{k}"][g, :, :])
                    w[nm] = t
                bt = wpool.tile([128, 15], F32, tag="bias")
                nc.sync.dma_start(bt[:], ins[f"bias{k}"])
                w["bias"] = bt
                return w

            def stage(k, intra, xsl_cur, xall):
                w = load_weights(k)

                # ---- K/V source selection ----
                if k == 0:
                    xsrc = xsrc_in
                else:
                    a_col, b_col = (0, 1) if intra else (2, 3)
                    xsrc = xpool.tile([128, 3, N], SD, tag="xsrc")
                    tmp = xpool.tile([128, 3, N], SD, tag="xsrctmp")
                    nc.vector.tensor_scalar_mul(
                        xsrc[:], xall[:, :, 0:N], selt[:, a_col:a_col + 1])
                    nc.vector.tensor_scalar_mul(
                        tmp[:], xall[:, :, N:2 * N], selt[:, b_col:b_col + 1])
                    nc.vector.tensor_tensor(
                        out=xsrc[:], in0=xsrc[:], in1=tmp[:], op=mybir.AluOpType.add)

                # ---- projections ----
                # K^T [per-head 300 x N], bias per-partition on copy-out
                kT = proj.tile([128, H, 3, N], SD, tag="kT")
                for h in range(H):
                    for t, (tg, tcnt) in enumerate(MT):
                        ps = ps_pj.tile([128, N], F32, tag="pj")
                        for ci, (g, cnt) in enumerate(CH):
                            nc.tensor.matmul(
                                ps[0:tcnt, :],
                                lhsT=w["wk"][0:cnt, g, h * D + t * 128: h * D + t * 128 + tcnt],
                                rhs=xsrc[0:cnt, g, :],
                                start=(ci == 0), stop=(ci == len(CH) - 1))
                        nc.scalar.activation(
                            kT[0:tcnt, h, t, :], ps[0:tcnt, :],
                            mybir.ActivationFunctionType.Identity,
                            bias=w["bias"][0:tcnt, 6 + h * 3 + t: 7 + h * 3 + t])
                # V plain, per head per j-tile [128 x 300] (no bias; bv folded into out bias)
                vst = proj.tile([128, H, 3, D], SD, tag="vst")
                for h in range(H):
                    for jt in range(3):
                        ps = ps_pj.tile([128, D], F32, tag="pj")
                        for ci, (g, cnt) in enumerate(CH):
                            nc.tensor.matmul(
                                ps[:, :],
                                lhsT=xsrc[0:cnt, g, jt * 128:(jt + 1) * 128],
                                rhs=w["wv"][0:cnt, g, h * D:(h + 1) * D],
                                start=(ci == 0), stop=(ci == len(CH) - 1))
                        nc.scalar.copy(vst[:, h, jt, :], ps[:, :])
                # Q^T slice [per-head 300 x SL], scale+bias folded host-side
                qT = proj.tile([128, H, 3, SL], SD, tag="qT")
                for h in range(H):
                    for t, (tg, tcnt) in enumerate(MT):
                        ps = ps_pj.tile([128, SL], F32, tag="pj")
                        for ci, (g, cnt) in enumerate(CH):
                            nc.tensor.matmul(
                                ps[0:tcnt, :],
                                lhsT=w["wq"][0:cnt, g, h * D + t * 128: h * D + t * 128 + tcnt],
                                rhs=xsl_cur[0:cnt, g, :],
                                start=(ci == 0), stop=(ci == len(CH) - 1))
                        nc.scalar.activation(
                            qT[0:tcnt, h, t, :], ps[0:tcnt, :],
                            mybir.ActivationFunctionType.Identity,
                            bias=w["bias"][0:tcnt, h * 3 + t: h * 3 + t + 1])

                # ---- attention per head ----
                alphaT = attn.tile([128, H, 3, SL], SD, tag="alphaT")
                for h in range(H):
                    lps = ps_lg.tile([SL, N], F32, tag="lg")
                    for t, (tg, tcnt) in enumerate(MT):
                        nc.tensor.matmul(
                            lps[:, :],
                            lhsT=qT[0:tcnt, h, t, :],
                            rhs=kT[0:tcnt, h, t, :],
                            start=(t == 0), stop=(t == len(MT) - 1))
                    if intra:
                        lm = attn.tile([SL, N], F32, tag="lm")
                        nc.vector.tensor_tensor(
                            out=lm[:], in0=lps[:], in1=maddt[:], op=mybir.AluOpType.add)
                        src = lm
                    else:
                        src = lps
                    negm = stats.tile([SL, 1], F32, tag="negm")
                    nc.vector.reduce_max(
                        out=negm[:], in_=src[:], axis=mybir.AxisListType.X, negate=True)
                    e = attn.tile([SL, N], SD, tag="e")
                    s = stats.tile([SL, 1], F32, tag="s")
                    nc.scalar.activation(
                        e[:], src[:], mybir.ActivationFunctionType.Exp,
                        bias=negm[:, 0:1], scale=1.0, accum_out=s[:, 0:1])
                    nc.vector.tensor_scalar_max(s[:], s[:], 1e-30)
                    rs = stats.tile([SL, 1], F32, tag="rs")
                    nc.vector.reciprocal(rs[:], s[:])
                    rs2 = stats.tile([SL, 1], F32, tag="rs2")
                    if intra:
                        nc.vector.tensor_tensor(
                            out=rs2[:], in0=rs[:], in1=hval[:], op=mybir.AluOpType.mult)
                    else:
                        nc.vector.tensor_scalar_mul(rs2[:], rs[:], 0.5)
                    alpha = attn.tile([SL, N], SD, tag="alpha")
                    nc.vector.tensor_scalar_mul(alpha[:], e[:], rs2[:, 0:1])
                    for jc in range(3):
                        tps = ps_tp.tile([128, SL], SD, tag="tp")
                        nc.tensor.transpose(
                            tps[:, :], alpha[:, jc * 128:(jc + 1) * 128], ident[:])
                        nc.vector.tensor_copy(alphaT[:, h, jc, :], tps[:, :])

                # ---- output accumulation: 0.5*(AV_h0+AV_h1) + skip + bias ----
                ops = ps_op.tile([128, 3, SL], F32, tag="op")
                for t, (tg, tcnt) in enumerate(MT):
                    nmm = H * 3 + len(CH)
                    i = 0
                    for h in range(H):
                        for jc in range(3):
                            nc.tensor.matmul(
                                ops[0:tcnt, t, :],
                                lhsT=vst[:, h, jc, t * 128:t * 128 + tcnt],
                                rhs=alphaT[:, h, jc, :],
                                start=(i == 0), stop=(i == nmm - 1))
                            i += 1
                    for (g, cnt) in CH:
                        nc.tensor.matmul(
                            ops[0:tcnt, t, :],
                            lhsT=w["ws"][0:cnt, g, t * 128:t * 128 + tcnt],
                            rhs=xsl_cur[0:cnt, g, :],
                            start=(i == 0), stop=(i == nmm - 1))
                        i += 1

                # add skip-bias, leaky relu = max(x+b, 0.01(x+b))
                xsl_new = xpool.tile([128, 3, SL], SD, tag="xsl")
                # zero the d-padding rows of the last chunk (shipped via AG)
                for p0 in (32, 64, 96):
                    nc.vector.memset(xsl_new[p0:p0 + 32, 2, :], 0.0)
                for t, (tg, tcnt) in enumerate(MT):
                    bcol = w["bias"][0:tcnt, 12 + t:13 + t]
                    xb = attn.tile([128, SL], F32, tag="xb")
                    lk = attn.tile([128, SL], F32, tag="lk")
                    nc.vector.tensor_scalar_add(xb[0:tcnt, :], ops[0:tcnt, t, :], bcol)
                    nc.vector.tensor_scalar(
                        out=lk[0:tcnt, :], in0=ops[0:tcnt, t, :],
                        scalar1=bcol, scalar2=0.01,
                        op0=mybir.AluOpType.add, op1=mybir.AluOpType.mult)
                    nc.vector.tensor_tensor(
                        out=xsl_new[0:tcnt, t, :], in0=xb[0:tcnt, :],
                        in1=lk[0:tcnt, :], op=mybir.AluOpType.max)

                # ---- AllGather ----
                agin = dram.tile([3, 128, SL], SD, tag="agin")
                for g in range(3):
                    nc.sync.dma_start(agin[g, :, :], xsl_new[:, g, :])
                agout = dram.tile([NC, 3, 128, SL], SD, tag="agout")
                nc.gpsimd.collective_compute(
                    "AllGather", mybir.AluOpType.bypass,
                    replica_groups=[list(range(NC))],
                    ins=[agin[:].opt()], outs=[agout[:].opt()])
                xall_new = xpool.tile([128, 3, 2 * N], SD, tag="xall")
                for b in range(NC):
                    for g in range(3):
                        nc.sync.dma_start(
                            xall_new[:, g, b * SL:(b + 1) * SL],
                            agout[b, g, :, :])
                return xsl_new, xall_new

            for k in range(6):
                xsl_cur, xall = stage(k, intra=(k % 2 == 0), xsl_cur=xsl_cur, xall=xall)

            # ---- pooling + MLP (replicated) ----
            psum1 = stats.tile([128, 3, 1], F32, tag="p1")
            psum2 = stats.tile([128, 3, 1], F32, tag="p2")
            nc.vector.reduce_sum(
                out=psum1[:], in_=xall[:, :, 0:N], axis=mybir.AxisListType.X)
            nc.vector.reduce_sum(
                out=psum2[:], in_=xall[:, :, N:2 * N], axis=mybir.AxisListType.X)
            p1t = stats.tile([128, 3, 1], F32, tag="p1s")
            p2t = stats.tile([128, 3, 1], F32, tag="p2s")
            nc.vector.tensor_scalar_mul(p1t[:], psum1[:], 1.0 / N)
            nc.vector.tensor_scalar_mul(p2t[:], psum2[:], 1.0 / N)

            mw1a = cpool.tile([128, 3, D], F32, tag="mw1a")
            for g in range(3):
                nc.sync.dma_start(mw1a[:, g, :], ins["mw1a"][g, :, :])
            mw1b = cpool.tile([128, 3, D], F32, tag="mw1b")
            for g in range(3):
                nc.sync.dma_start(mw1b[:, g, :], ins["mw1b"][g, :, :])
            mw2 = cpool.tile([128, 3, D], F32, tag="mw2")
            for g in range(3):
                nc.sync.dma_start(mw2[:, g, :], ins["mw2"][g, :, :])
            mw3 = cpool.tile([128, 3, 1], F32, tag="mw3")
            for g in range(3):
                nc.sync.dma_start(mw3[:, g, :], ins["mw3"][g, :, :])
            mb1 = cpool.tile([128, 3], F32, tag="mb1")
            nc.sync.dma_start(mb1[:], ins["mb1"])
            mb2 = cpool.tile([128, 3], F32, tag="mb2")
            nc.sync.dma_start(mb2[:], ins["mb2"])
            mb3 = cpool.tile([1, 1], F32, tag="mb3")
            nc.sync.dma_start(mb3[:], ins["mb3"])

            def mlp_layer(wa, wb, xa, xb_, bias, tag):
                # h[t-slice] = leaky(wa.T@xa (+ wb.T@xb_) + bias[:, t])
                ht = stats.tile([128, 3, 1], F32, tag=tag)
                for t, (tg, tcnt) in enumerate(MT):
                    ps = ps_pj.tile([128, 1], F32, tag="pj")
                    n_mm = len(CH) * (2 if wb is not None else 1)
                    i = 0
                    for (g, cnt) in CH:
                        nc.tensor.matmul(
                            ps[0:tcnt, :],
                            lhsT=wa[0:cnt, g, t * 128:t * 128 + tcnt],
                            rhs=xa[0:cnt, g, :],
                            start=(i == 0), stop=(i == n_mm - 1))
                        i += 1
                    if wb is not None:
                        for (g, cnt) in CH:
                            nc.tensor.matmul(
                                ps[0:tcnt, :],
                                lhsT=wb[0:cnt, g, t * 128:t * 128 + tcnt],
                                rhs=xb_[0:cnt, g, :],
                                start=(i == 0), stop=(i == n_mm - 1))
                            i += 1
                    bcol = bias[0:tcnt, t:t + 1]
                    hb = stats.tile([128, 1], F32, tag="hb")
                    lk2 = stats.tile([128, 1], F32, tag="lk2")
                    nc.vector.tensor_scalar_add(hb[0:tcnt, :], ps[0:tcnt, :], bcol)
                    nc.vector.tensor_scalar(
                        out=lk2[0:tcnt, :], in0=ps[0:tcnt, :],
                        scalar1=bcol, scalar2=0.01,
                        op0=mybir.AluOpType.add, op1=mybir.AluOpType.mult)
                    nc.vector.tensor_tensor(
                        out=ht[0:tcnt, t, :], in0=hb[0:tcnt, :], in1=lk2[0:tcnt, :],
                        op=mybir.AluOpType.max)
                return ht

            h1 = mlp_layer(mw1a, mw1b, p1t, p2t, mb1, "h1")
            h2 = mlp_layer(mw2, None, h1, None, mb2, "h2")
            sps = ps_pj.tile([1, 1], F32, tag="pj")
            for ci, (g, cnt) in enumerate(CH):
                nc.tensor.matmul(
                    sps[0:1, :], lhsT=mw3[0:cnt, g, 0:1], rhs=h2[0:cnt, g, :],
                    start=(ci == 0), stop=(ci == len(CH) - 1))
            so = stats.tile([1, 1], F32, tag="so")
            nc.scalar.activation(so[:], sps[:], mybir.ActivationFunctionType.Sigmoid,
                                 bias=mb3[0:1, 0:1])

            nc.sync.dma_start(p1o, p1t[:, :, 0])
            nc.sync.dma_start(p2o, p2t[:, :, 0])
            nc.sync.dma_start(sigo, so[:])

    nc.compile()
    return nc


# ---------------- host-side packing ----------------

def _pack_w(W, scale=1.0):
    """W [300,F] -> [3,128,F] float32 (d-chunked on partitions)."""
    F = W.shape[1]
    out = np.zeros((3, 128, F), np.float32)
    Ws = W * scale
    for g, cnt in CH:
        out[g, :cnt] = Ws[g * 128:g * 128 + cnt]
    return out


def _pack_col(v):
    """v [<=600] -> per-partition column pack [128, ntiles] (f-chunked)."""
    nt = (len(v) + 127) // 128
    out = np.zeros((128, nt), np.float32)
    for t in range(nt):
        seg = v[t * 128:(t + 1) * 128]
        out[:len(seg), t] = seg
    return out


def _pack_xT(x):
    """x [384,300] -> x^T packed [3,128,384]."""
    out = np.zeros((3, 128, N), np.float32)
    xT = x.T
    for g, cnt in CH:
        out[g, :cnt] = xT[g * 128:g * 128 + cnt]
    return out


def _np_sd(mode):
    return mybir.dt.np(_sd(mode))


def _host_pack(inputs, mode):
    sd = _np_sd(mode)
    x1 = np.asarray(inputs["x_1"], np.float32)
    x2 = np.asarray(inputs["x_2"], np.float32)
    ei1 = np.asarray(inputs["edge_idx_1"]).astype(np.int64)
    ei2 = np.asarray(inputs["edge_idx_2"]).astype(np.int64)

    def conv_pack(prefix):
        g = lambda nm: np.asarray(inputs[prefix + nm], np.float32)
        out = []
        for layer in range(NLAYERS):
            Wq, bq = g("Wq")[layer], g("bq")[layer]
            Wk, bk = g("Wk")[layer], g("bk")[layer]
            Wv, bv = g("Wv")[layer], g("bv")[layer]
            Ws, bs = g("Ws")[layer], g("bs")[layer]
            bias_out = 0.5 * (bv[:D] + bv[D:]) + bs
            bias = np.zeros((128, 15), np.float32)
            # bq/bk per-head per-M-tile columns (head-aligned 3 tiles of 300)
            for h in range(H):
                bh = _pack_col(bq[h * D:(h + 1) * D] * SCALE)
                bias[:, h * 3:h * 3 + bh.shape[1]] = bh
                kh = _pack_col(bk[h * D:(h + 1) * D])
                bias[:, 6 + h * 3:6 + h * 3 + kh.shape[1]] = kh
            bo = _pack_col(bias_out)
            bias[:, 12:12 + bo.shape[1]] = bo
            out.append({
                "wq": _pack_w(Wq, SCALE).astype(sd),
                "wk": _pack_w(Wk).astype(sd),
                "wv": _pack_w(Wv).astype(sd),
                "ws": _pack_w(Ws).astype(sd),
                "bias": bias,
            })
        return out

    packs = {p: conv_pack(p + "_") for p in ("tsa", "gsa", "tca", "gca")}

    def mask_pack(ei, lo, hi):
        C = np.zeros((N, N), np.float64)
        np.add.at(C, (ei[1], ei[0]), 1.0)   # rows = dst, cols = src
        Cs = C[lo:hi]
        madd = np.where(Cs > 0, np.log(np.maximum(Cs, 1.0)), NEG).astype(np.float32)
        hvalid = (0.5 * (Cs.sum(axis=1) > 0)).astype(np.float32).reshape(SL, 1)
        return madd, hvalid

    mlp = {
        "mw1a": _pack_w(np.asarray(inputs["mlp_W1"], np.float32)[:D]),
        "mw1b": _pack_w(np.asarray(inputs["mlp_W1"], np.float32)[D:]),
        "mw2": _pack_w(np.asarray(inputs["mlp_W2"], np.float32)),
        "mw3": _pack_w(np.asarray(inputs["mlp_W3"], np.float32)),
        "mb1": _pack_col(np.asarray(inputs["mlp_b1"], np.float32)),
        "mb2": _pack_col(np.asarray(inputs["mlp_b2"], np.float32)),
        "mb3": np.asarray(inputs["mlp_b3"], np.float32).reshape(1, 1),
    }

    in_maps = []
    for c in range(NC):
        own1 = c < 4
        sl = c % 4
        x = x1 if own1 else x2
        ei = ei1 if own1 else ei2
        intra_p, cross_p = (("tsa", "tca") if own1 else ("gsa", "gca"))
        m = {}
        for k in range(6):
            pk = packs[intra_p if k % 2 == 0 else cross_p][k // 2]
            for nm in ("wq", "wk", "wv", "ws"):
                m[f"{nm}{k}"] = pk[nm]
            m[f"bias{k}"] = pk["bias"]
        xp = _pack_xT(x).astype(sd)
        m["xsrc0"] = xp
        m["xsl0"] = np.ascontiguousarray(xp[:, :, sl * SL:(sl + 1) * SL])
        madd, hvalid = mask_pack(ei, sl * SL, (sl + 1) * SL)
        m["madd"] = madd
        m["hvalid"] = hvalid
        sel = np.zeros((128, 4), np.float32)
        sel[:, 0] = 1.0 if own1 else 0.0     # own half = A (graph1 blocks)
        sel[:, 1] = 0.0 if own1 else 1.0
        sel[:, 2] = 0.0 if own1 else 1.0     # other half
        sel[:, 3] = 1.0 if own1 else 0.0
        m["sel"] = sel
        m["ident"] = np.eye(SL, dtype=np.float32).astype(sd)
        m.update(mlp)
        in_maps.append(m)
    return in_maps


# ---------------- cached runner (axon PJRT, jit reused across calls) ----------------

_PROG = {}
_RUNNER = {}


def _get_runner(mode):
    if mode in _RUNNER:
        return _RUNNER[mode]
    if mode not in _PROG:
        _PROG[mode] = _build_program(mode)
    nc = _PROG[mode]

    import jax
    from jax.sharding import Mesh, PartitionSpec
    from jax.experimental.shard_map import shard_map
    from concourse import bass2jax

    bass2jax.install_neuronx_cc_hook()

    partition_name = (
        nc.partition_id_tensor.name if nc.partition_id_tensor else None)
    in_names, out_names, out_avals, zero_shapes = [], [], [], []
    for alloc in nc.m.functions[0].allocations:
        if not isinstance(alloc, mybir.MemoryLocationSet):
            continue
        name = alloc.memorylocations[0].name
        if alloc.kind == "ExternalInput":
            if name != partition_name:
                in_names.append(name)
        elif alloc.kind == "ExternalOutput":
            out_names.append(name)
            shape = tuple(alloc.tensor_shape)
            dtype = mybir.dt.np(alloc.dtype)
            out_avals.append(jax.core.ShapedArray(shape, dtype))
            zero_shapes.append((shape, dtype))
    n_params = len(in_names)
    n_outs = len(out_names)
    all_names = in_names + out_names
    if partition_name is not None:
        all_names = all_names + [partition_name]

    def _body(*args):
        operands = list(args)
        if partition_name is not None:
            operands.append(bass2jax.partition_id_tensor())
        outs = bass2jax._bass_exec_p.bind(
            *operands,
            out_avals=tuple(out_avals),
            in_names=tuple(all_names),
            out_names=tuple(out_names),
            lowering_input_output_aliases=(),
            sim_require_finite=True,
            sim_require_nnan=True,
            nc=nc,
        )
        return tuple(outs)

    devices = jax.devices()[:NC]
    mesh = Mesh(np.asarray(devices), ("core",))
    donate = tuple(range(n_params, n_params + n_outs))
    sharded = jax.jit(
        shard_map(_body, mesh=mesh,
                  in_specs=(PartitionSpec("core"),) * (n_params + n_outs),
                  out_specs=(PartitionSpec("core"),) * n_outs,
                  check_rep=False),
        donate_argnums=donate, keep_unused=True)

    def run(in_maps):
        concat_in = [
            np.concatenate([np.asarray(in_maps[c][nm]) for c in range(NC)], axis=0)
            for nm in in_names
        ]
        concat_zeros = [
            np.zeros((NC * s[0], *s[1:]), dt) for (s, dt) in zero_shapes
        ]
        out_arrs = sharded(*concat_in, *concat_zeros)
        core0 = {
            nm: np.asarray(out_arrs[i]).reshape(NC, *out_avals[i].shape)[0]
            for i, nm in enumerate(out_names)
        }
        return core0

    _RUNNER[mode] = run
    return run


def kernel(**inputs):
    run = _get_runner(MODE)
    in_maps = _host_pack(inputs, MODE)
    out = run(in_maps)
    p1 = out["p1o"].T.reshape(-1)[:D].astype(np.float32)
    p2 = out["p2o"].T.reshape(-1)[:D].astype(np.float32)
    sig = out["sigo"].reshape(1).astype(np.float32)
    return p1, p2, sig
